# revision 15
# baseline (speedup 1.0000x reference)
"""Trainium2 Bass kernel for nn_GatedShortBlock (gated depthwise-conv block).

Math (per batch b):
  BCx = x @ w1.T ; Bg, Cg, Xg = split(BCx, 3)
  gated = Bg * Xg
  conv  = causal depthwise conv1d(gated, conv_w, K=4)  (left pad 3)
  out   = (Cg * conv) @ w2.T

Sharding: data-parallel over (batch, seq-half) -> 8 shards of 2048 tokens.
Each core computes its shard fully on-device in a channel-major (transposed)
layout; the 3-token causal halo of `gated` at each shard start is computed
on the host (tiny) and shipped as an input.

v2: fp16 operands (full PE rate, ~6e-4 rel err), single pass over w1/w2
(each weight tile is loaded once and reused for all 4 token chunks),
whole 2048-token shard processed as one block.
"""

import sys

sys.path.insert(0, "/opt/trn_rl_repo")

import numpy as np
from contextlib import ExitStack

import concourse.bass as bass
import concourse.tile as tile
from concourse import bacc, mybir
from concourse.bass_utils import run_bass_kernel_spmd

F32 = mybir.dt.float32
F16 = mybir.dt.float16
KS = 4  # conv kernel size
KG = 4  # k-subtiles batched per weight DMA

D = 2048
E = 3 * D
T = 2048  # tokens per core shard
CH = 512  # psum chunk width
ND = D // 128  # contraction tiles
NC = D // 128  # channel tiles
NCH = T // CH  # chunks per shard


def build_program():
    """One-core program; SPMD across cores with different data."""
    nc = bacc.Bacc(None)
    xT = nc.dram_tensor("xT", [D, T], F16, kind="ExternalInput")
    w1T = nc.dram_tensor("w1T", [D, E], F16, kind="ExternalInput")
    w2T = nc.dram_tensor("w2T", [D, D], F16, kind="ExternalInput")
    cw = nc.dram_tensor("cw", [D, KS], F32, kind="ExternalInput")
    gh = nc.dram_tensor("gh", [D, KS - 1], F16, kind="ExternalInput")
    outT = nc.dram_tensor("outT", [D, T], F32, kind="ExternalOutput")

    def w_batch_src(w, ncols, e, g):
        # [128 part, KG ksub, 128 m] gather of KG stacked [128,128] tiles:
        # element (p, ks, m) = w[(g*KG+ks)*128 + p, e*128 + m]
        off = (g * KG * 128) * ncols + e * 128
        return bass.AP(w, off, [[ncols, 128], [128 * ncols, KG], [1, 128]])

    with tile.TileContext(nc) as tc, ExitStack() as ctx:
        wp = ctx.enter_context(tc.tile_pool(name="wp", bufs=24))
        w2p = ctx.enter_context(tc.tile_pool(name="w2p", bufs=8))
        xp = ctx.enter_context(tc.tile_pool(name="xp", bufs=1))
        rp = ctx.enter_context(tc.tile_pool(name="rp", bufs=1))
        gwp = ctx.enter_context(tc.tile_pool(name="gwp", bufs=3))
        scrp = ctx.enter_context(tc.tile_pool(name="scrp", bufs=6))
        tmpp = ctx.enter_context(tc.tile_pool(name="tmpp", bufs=4))
        stgp = ctx.enter_context(tc.tile_pool(name="stgp", bufs=4))
        smallp = ctx.enter_context(tc.tile_pool(name="smallp", bufs=1))
        psp = ctx.enter_context(tc.tile_pool(name="psp", bufs=8, space="PSUM"))

        # persistent small tiles: conv weights (GpSimd queue: off the
        # critical weight/x streams)
        cwt = []
        for c in range(NC):
            t = smallp.tile([128, KS], F32, tag=f"cw{c}", name=f"cw{c}")
            nc.gpsimd.dma_start(t[:], cw[c * 128 : (c + 1) * 128, :])
            cwt.append(t)

        # x resident in SBUF (fp16, 64KB/partition) on the Scalar queue,
        # gathered KG k-planes per DMA (few large issues -> fast startup),
        # split by 512-col chunk so chunk u=0 lands first.
        # xu[u][kk][:, j*CH:(j+1)*CH] holds x k-tile (kk*KG+j), cols of chunk u.
        xu = [
            [
                xp.tile([128, KG * CH], F16, tag=f"x{u}_{kk}", name=f"x{u}_{kk}")
                for kk in range(ND // KG)
            ]
            for u in range(NCH)
        ]
        def load_x_chunk(u, eng):
            for kk in range(ND // KG):
                srcap = bass.AP(
                    xT,
                    (kk * KG * 128) * T + u * CH,
                    [[T, 128], [128 * T, KG], [1, CH]],
                )
                eng.dma_start(
                    xu[u][kk][:].rearrange("p (g m) -> p g m", m=CH), srcap
                )

        # u0 first (PE-critical), u1/u3 behind it on the Scalar queue;
        # u2 goes on the Sync queue between c0's weight loads (see c-loop).
        for u in (0, 1, 3):
            load_x_chunk(u, nc.scalar)

        def load_w_tiles(pool, tag, w, ncols, e):
            tiles = []
            for g in range(ND // KG):
                wt = pool.tile([128, KG * 128], F16, tag=tag, name=f"{tag}_t")
                nc.sync.dma_start(
                    wt[:].rearrange("p (g m) -> p g m", m=128),
                    w_batch_src(w, ncols, e, g),
                )
                tiles.append(wt)
            return tiles

        def mm_accum(ps, wtiles, u):
            for g in range(ND // KG):
                for ks in range(KG):
                    k = g * KG + ks
                    nc.tensor.matmul(
                        ps[:],
                        wtiles[g][:, ks * 128 : (ks + 1) * 128],
                        xu[u][g][:, ks * CH : (ks + 1) * CH],
                        start=(k == 0),
                        stop=(k == ND - 1),
                    )

        Rt = []
        for c in range(NC):
            wB = load_w_tiles(wp, "w1", w1T, E, c)
            wX = load_w_tiles(wp, "w1", w1T, E, 2 * NC + c)
            if c == 0:
                # c0 streams x while computing: u2 rides the Sync queue
                # between the weight loads it is needed after.
                load_x_chunk(2, nc.sync)
            wC = load_w_tiles(wp, "w1", w1T, E, NC + c)
            gw = gwp.tile([128, T + KS - 1], F16, tag="gw", name=f"gw{c}")
            nc.gpsimd.dma_start(gw[:, 0 : KS - 1], gh[c * 128 : (c + 1) * 128, :])
            R = rp.tile([128, T], F16, tag=f"R{c}", name=f"R{c}")
            Rt.append(R)

            def conv_chunk(u, s_dst):
                # conv over gw cols [u*CH, (u+1)*CH): reads gw[:, u*CH+j :
                # u*CH+j+CH] for taps j=0..3 (3-col halo from chunk u-1).
                t0 = scrp.tile([128, CH], F16, tag="scc", name=f"scc{c}_{u}_0")
                nc.vector.tensor_scalar_mul(
                    t0[:], gw[:, u * CH : u * CH + CH], cwt[c][:, 0:1]
                )
                prev = t0
                for j in range(1, KS):
                    dst = (
                        s_dst
                        if j == KS - 1
                        else scrp.tile(
                            [128, CH], F16, tag="scc", name=f"scc{c}_{u}_{j}"
                        )
                    )
                    nc.vector.scalar_tensor_tensor(
                        dst[:],
                        gw[:, u * CH + j : u * CH + j + CH],
                        cwt[c][:, j : j + 1],
                        prev[:],
                        mybir.AluOpType.mult,
                        mybir.AluOpType.add,
                    )
                    prev = dst
                return prev

            for u in range(NCH):
                psB = psp.tile([128, CH], F32, tag="ps", name=f"psB{c}_{u}")
                mm_accum(psB, wB, u)
                psX = psp.tile([128, CH], F32, tag="ps", name=f"psX{c}_{u}")
                mm_accum(psX, wX, u)
                # DVE reads at most one PSUM operand per instruction:
                # stage Bg into SBUF, then multiply with Xg.
                tmp = tmpp.tile([128, CH], F32, tag="tmp", name=f"tmp{c}_{u}")
                nc.vector.tensor_copy(tmp[:], psB[:])
                nc.vector.tensor_mul(
                    gw[:, KS - 1 + u * CH : KS - 1 + (u + 1) * CH], tmp[:], psX[:]
                )
                # chunk-major: each x chunk feeds B, X and C back-to-back
                # (keeps c0's startup x-stream demand low), and the conv +
                # R-mul run per chunk so the psC bank frees immediately.
                psC = psp.tile([128, CH], F32, tag="ps", name=f"psC{c}_{u}")
                mm_accum(psC, wC, u)
                sc = scrp.tile([128, CH], F16, tag="scc", name=f"sfin{c}_{u}")
                conv_chunk(u, sc)
                nc.vector.tensor_mul(
                    R[:, u * CH : (u + 1) * CH], sc[:], psC[:]
                )

        # ---- mm2: out = R.T @ w2.T (channel-major) ----
        for f in range(NC):
            w2t = load_w_tiles(w2p, "w2", w2T, D, f)
            for u in range(NCH):
                ps2 = psp.tile([128, CH], F32, tag="ps", name=f"ps2{f}_{u}")
                for g in range(NC // KG):
                    for cs in range(KG):
                        c = g * KG + cs
                        nc.tensor.matmul(
                            ps2[:],
                            w2t[g][:, cs * 128 : (cs + 1) * 128],
                            Rt[c][:, u * CH : (u + 1) * CH],
                            start=(c == 0),
                            stop=(c == NC - 1),
                        )
                st = stgp.tile([128, CH], F32, tag="stg", name=f"st{f}_{u}")
                nc.vector.tensor_copy(st[:], ps2[:])
                nc.sync.dma_start(
                    outT[f * 128 : (f + 1) * 128, u * CH : (u + 1) * CH],
                    st[:],
                )

    nc.finalize()
    return nc


def shard_inputs(x, w1, w2, conv_w):
    """Full inputs -> per-core in_maps (channel-major fp16 device layouts)."""
    B, S, _ = x.shape
    n_shards = (B * S) // T
    w1T = np.ascontiguousarray(w1.T).astype(np.float16)
    w2T = np.ascontiguousarray(w2.T).astype(np.float16)
    cwh = np.ascontiguousarray(conv_w[:, 0, :]).astype(np.float32)
    shards_per_batch = S // T
    in_maps = []
    for s in range(n_shards):
        b, h = divmod(s, shards_per_batch)
        xs = x[b, h * T : (h + 1) * T, :]
        xTs = np.ascontiguousarray(xs.T).astype(np.float16)
        if h == 0:
            ghs = np.zeros((D, KS - 1), np.float16)
        else:
            xh = x[b, h * T - (KS - 1) : h * T, :]
            Bg = xh @ w1[0:D].T
            Xg = xh @ w1[2 * D : 3 * D].T
            ghs = np.ascontiguousarray((Bg * Xg).T).astype(np.float16)
        in_maps.append({"xT": xTs, "w1T": w1T, "w2T": w2T, "cw": cwh, "gh": ghs})
    return in_maps


_PROGRAM_CACHE = {}


def run(x, w1, w2, conv_w, trace=False, **_ignored):
    B, S, _ = x.shape
    if "prog" not in _PROGRAM_CACHE:
        _PROGRAM_CACHE["prog"] = build_program()
    nc = _PROGRAM_CACHE["prog"]
    in_maps = shard_inputs(x, w1, w2, conv_w)
    n_shards = len(in_maps)
    res = run_bass_kernel_spmd(nc, in_maps, core_ids=list(range(n_shards)), trace=trace)
    shards_per_batch = S // T
    out = np.empty((B, S, D), np.float32)
    for s in range(n_shards):
        b, h = divmod(s, shards_per_batch)
        out[b, h * T : (h + 1) * T, :] = res.results[s]["outT"].T
    return out, res


def kernel(x, w1, w2, conv_w):
    x = np.asarray(x, np.float32)
    w1 = np.asarray(w1, np.float32)
    w2 = np.asarray(w2, np.float32)
    conv_w = np.asarray(conv_w, np.float32)
    out, _ = run(x, w1, w2, conv_w)
    return out


# revision 18
# speedup vs baseline: 1.0873x; 1.0873x over previous
"""Trainium2 Bass kernel for nn_GatedShortBlock (gated depthwise-conv block).

Math (per batch b):
  BCx = x @ w1.T ; Bg, Cg, Xg = split(BCx, 3)
  gated = Bg * Xg
  conv  = causal depthwise conv1d(gated, conv_w, K=4)  (left pad 3)
  out   = (Cg * conv) @ w2.T

Sharding: data-parallel over (batch, seq-half) -> 8 shards of 2048 tokens.
Each core computes its shard fully on-device in a channel-major (transposed)
layout; the 3-token causal halo of `gated` at each shard start is computed
on the host (tiny) and shipped as an input.

v2: fp16 operands (full PE rate, ~6e-4 rel err), single pass over w1/w2
(each weight tile is loaded once and reused for all 4 token chunks),
whole 2048-token shard processed as one block.
"""

import sys

sys.path.insert(0, "/opt/trn_rl_repo")

import numpy as np
from contextlib import ExitStack

import concourse.bass as bass
import concourse.tile as tile
from concourse import bacc, mybir
from concourse.bass_utils import run_bass_kernel_spmd

F32 = mybir.dt.float32
F16 = mybir.dt.float16
KS = 4  # conv kernel size
KG = 4  # k-subtiles batched per weight DMA

D = 2048
E = 3 * D
T = 2048  # tokens per core shard
CH = 512  # psum chunk width
ND = D // 128  # contraction tiles
NC = D // 128  # channel tiles
NCH = T // CH  # chunks per shard


def build_program():
    """One-core program; SPMD across cores with different data."""
    nc = bacc.Bacc(None)
    xT = nc.dram_tensor("xT", [D, T], F16, kind="ExternalInput")
    w1T = nc.dram_tensor("w1T", [D, E], F16, kind="ExternalInput")
    w2T = nc.dram_tensor("w2T", [D, D], F16, kind="ExternalInput")
    cw = nc.dram_tensor("cw", [D, KS], F32, kind="ExternalInput")
    gh = nc.dram_tensor("gh", [D, KS - 1], F16, kind="ExternalInput")
    outT = nc.dram_tensor("outT", [D, T], F32, kind="ExternalOutput")

    def w_batch_src(w, ncols, e, g):
        # [128 part, KG ksub, 128 m] gather of KG stacked [128,128] tiles:
        # element (p, ks, m) = w[(g*KG+ks)*128 + p, e*128 + m]
        off = (g * KG * 128) * ncols + e * 128
        return bass.AP(w, off, [[ncols, 128], [128 * ncols, KG], [1, 128]])

    with tile.TileContext(nc) as tc, ExitStack() as ctx:
        wp = ctx.enter_context(tc.tile_pool(name="wp", bufs=24))
        w2p = ctx.enter_context(tc.tile_pool(name="w2p", bufs=8))
        xp = ctx.enter_context(tc.tile_pool(name="xp", bufs=1))
        rp = ctx.enter_context(tc.tile_pool(name="rp", bufs=1))
        gwp = ctx.enter_context(tc.tile_pool(name="gwp", bufs=3))
        scrp = ctx.enter_context(tc.tile_pool(name="scrp", bufs=3))
        tmpp = ctx.enter_context(tc.tile_pool(name="tmpp", bufs=4))
        stgp = ctx.enter_context(tc.tile_pool(name="stgp", bufs=4))
        smallp = ctx.enter_context(tc.tile_pool(name="smallp", bufs=1))
        psp = ctx.enter_context(tc.tile_pool(name="psp", bufs=8, space="PSUM"))

        # persistent small tiles: conv weights (GpSimd queue: off the
        # critical weight/x streams)
        cwt = []
        for c in range(NC):
            t = smallp.tile([128, KS], F32, tag=f"cw{c}", name=f"cw{c}")
            nc.gpsimd.dma_start(t[:], cw[c * 128 : (c + 1) * 128, :])
            cwt.append(t)

        # x resident in SBUF (fp16, 64KB/partition) on the Scalar queue,
        # gathered KG k-planes per DMA (few large issues -> fast startup),
        # split by 512-col chunk so chunk u=0 lands first.
        # xu[u][kk][:, j*CH:(j+1)*CH] holds x k-tile (kk*KG+j), cols of chunk u.
        xu = [
            [
                xp.tile([128, KG * CH], F16, tag=f"x{u}_{kk}", name=f"x{u}_{kk}")
                for kk in range(ND // KG)
            ]
            for u in range(NCH)
        ]
        def load_x_chunk(u, eng):
            for kk in range(ND // KG):
                srcap = bass.AP(
                    xT,
                    (kk * KG * 128) * T + u * CH,
                    [[T, 128], [128 * T, KG], [1, CH]],
                )
                eng.dma_start(
                    xu[u][kk][:].rearrange("p (g m) -> p g m", m=CH), srcap
                )

        # u0 first (PE-critical), u2/u3 behind it on the Scalar queue;
        # u1 goes on the Sync queue right after c0's weights (see c-loop).
        for u in (0, 2, 3):
            load_x_chunk(u, nc.scalar)

        def load_w_tiles(pool, tag, w, ncols, e):
            tiles = []
            for g in range(ND // KG):
                wt = pool.tile([128, KG * 128], F16, tag=tag, name=f"{tag}_t")
                nc.sync.dma_start(
                    wt[:].rearrange("p (g m) -> p g m", m=128),
                    w_batch_src(w, ncols, e, g),
                )
                tiles.append(wt)
            return tiles

        def mm_accum(ps, wtiles, u):
            for g in range(ND // KG):
                for ks in range(KG):
                    k = g * KG + ks
                    nc.tensor.matmul(
                        ps[:],
                        wtiles[g][:, ks * 128 : (ks + 1) * 128],
                        xu[u][g][:, ks * CH : (ks + 1) * CH],
                        start=(k == 0),
                        stop=(k == ND - 1),
                    )

        Rt = []
        for c in range(NC):
            wB = load_w_tiles(wp, "w1", w1T, E, c)
            wX = load_w_tiles(wp, "w1", w1T, E, 2 * NC + c)
            wC = load_w_tiles(wp, "w1", w1T, E, NC + c)
            if c == 0:
                load_x_chunk(1, nc.sync)
            gw = gwp.tile([128, T + KS - 1], F16, tag="gw", name=f"gw{c}")
            nc.gpsimd.dma_start(gw[:, 0 : KS - 1], gh[c * 128 : (c + 1) * 128, :])
            for u in range(NCH):
                psB = psp.tile([128, CH], F32, tag="ps", name=f"psB{c}_{u}")
                mm_accum(psB, wB, u)
                psX = psp.tile([128, CH], F32, tag="ps", name=f"psX{c}_{u}")
                mm_accum(psX, wX, u)
                # DVE reads at most one PSUM operand per instruction:
                # stage Bg into SBUF, then multiply with Xg.
                tmp = tmpp.tile([128, CH], F32, tag="tmp", name=f"tmp{c}_{u}")
                nc.vector.tensor_copy(tmp[:], psB[:])
                nc.vector.tensor_mul(
                    gw[:, KS - 1 + u * CH : KS - 1 + (u + 1) * CH], tmp[:], psX[:]
                )
            # depthwise causal conv over gw -> s (fp16, full width)
            s = scrp.tile([128, T], F16, tag="scr", name=f"s0_{c}")
            nc.vector.tensor_scalar_mul(s[:], gw[:, 0:T], cwt[c][:, 0:1])
            for j in range(1, KS):
                s2 = scrp.tile([128, T], F16, tag="scr", name=f"s{j}_{c}")
                nc.vector.scalar_tensor_tensor(
                    s2[:],
                    gw[:, j : j + T],
                    cwt[c][:, j : j + 1],
                    s[:],
                    mybir.AluOpType.mult,
                    mybir.AluOpType.add,
                )
                s = s2
            # R = Cg * conv
            R = rp.tile([128, T], F16, tag=f"R{c}", name=f"R{c}")
            Rt.append(R)
            for u in range(NCH):
                psC = psp.tile([128, CH], F32, tag="ps", name=f"psC{c}_{u}")
                mm_accum(psC, wC, u)
                nc.vector.tensor_mul(
                    R[:, u * CH : (u + 1) * CH],
                    s[:, u * CH : (u + 1) * CH],
                    psC[:],
                )

        # ---- mm2: out = R.T @ w2.T (channel-major) ----
        for f in range(NC):
            w2t = load_w_tiles(w2p, "w2", w2T, D, f)
            for u in range(NCH):
                ps2 = psp.tile([128, CH], F32, tag="ps", name=f"ps2{f}_{u}")
                for g in range(NC // KG):
                    for cs in range(KG):
                        c = g * KG + cs
                        nc.tensor.matmul(
                            ps2[:],
                            w2t[g][:, cs * 128 : (cs + 1) * 128],
                            Rt[c][:, u * CH : (u + 1) * CH],
                            start=(c == 0),
                            stop=(c == NC - 1),
                        )
                st = stgp.tile([128, CH], F32, tag="stg", name=f"st{f}_{u}")
                nc.vector.tensor_copy(st[:], ps2[:])
                nc.sync.dma_start(
                    outT[f * 128 : (f + 1) * 128, u * CH : (u + 1) * CH],
                    st[:],
                )

    nc.finalize()
    return nc


def shard_inputs(x, w1, w2, conv_w):
    """Full inputs -> per-core in_maps (channel-major fp16 device layouts)."""
    B, S, _ = x.shape
    n_shards = (B * S) // T
    w1T = np.ascontiguousarray(w1.T).astype(np.float16)
    w2T = np.ascontiguousarray(w2.T).astype(np.float16)
    cwh = np.ascontiguousarray(conv_w[:, 0, :]).astype(np.float32)
    shards_per_batch = S // T
    in_maps = []
    for s in range(n_shards):
        b, h = divmod(s, shards_per_batch)
        xs = x[b, h * T : (h + 1) * T, :]
        xTs = np.ascontiguousarray(xs.T).astype(np.float16)
        if h == 0:
            ghs = np.zeros((D, KS - 1), np.float16)
        else:
            xh = x[b, h * T - (KS - 1) : h * T, :]
            Bg = xh @ w1[0:D].T
            Xg = xh @ w1[2 * D : 3 * D].T
            ghs = np.ascontiguousarray((Bg * Xg).T).astype(np.float16)
        in_maps.append({"xT": xTs, "w1T": w1T, "w2T": w2T, "cw": cwh, "gh": ghs})
    return in_maps


_PROGRAM_CACHE = {}


def run(x, w1, w2, conv_w, trace=False, **_ignored):
    B, S, _ = x.shape
    if "prog" not in _PROGRAM_CACHE:
        _PROGRAM_CACHE["prog"] = build_program()
    nc = _PROGRAM_CACHE["prog"]
    in_maps = shard_inputs(x, w1, w2, conv_w)
    n_shards = len(in_maps)
    res = run_bass_kernel_spmd(nc, in_maps, core_ids=list(range(n_shards)), trace=trace)
    shards_per_batch = S // T
    out = np.empty((B, S, D), np.float32)
    for s in range(n_shards):
        b, h = divmod(s, shards_per_batch)
        out[b, h * T : (h + 1) * T, :] = res.results[s]["outT"].T
    return out, res


def kernel(x, w1, w2, conv_w):
    x = np.asarray(x, np.float32)
    w1 = np.asarray(w1, np.float32)
    w2 = np.asarray(w2, np.float32)
    conv_w = np.asarray(conv_w, np.float32)
    out, _ = run(x, w1, w2, conv_w)
    return out


# revision 22
# speedup vs baseline: 1.0932x; 1.0055x over previous
"""Trainium2 Bass kernel for nn_GatedShortBlock (gated depthwise-conv block).

Math (per batch b):
  BCx = x @ w1.T ; Bg, Cg, Xg = split(BCx, 3)
  gated = Bg * Xg
  conv  = causal depthwise conv1d(gated, conv_w, K=4)  (left pad 3)
  out   = (Cg * conv) @ w2.T

Sharding: data-parallel over (batch, seq-half) -> 8 shards of 2048 tokens.
Each core computes its shard fully on-device in a channel-major (transposed)
layout; the 3-token causal halo of `gated` at each shard start is computed
on the host (tiny) and shipped as an input.

v2: fp16 operands (full PE rate, ~6e-4 rel err), single pass over w1/w2
(each weight tile is loaded once and reused for all 4 token chunks),
whole 2048-token shard processed as one block.
"""

import sys

sys.path.insert(0, "/opt/trn_rl_repo")

import numpy as np
from contextlib import ExitStack

import concourse.bass as bass
import concourse.tile as tile
from concourse import bacc, mybir
from concourse.bass_utils import run_bass_kernel_spmd

F32 = mybir.dt.float32
F16 = mybir.dt.float16
KS = 4  # conv kernel size
KG = 4  # k-subtiles batched per weight DMA

D = 2048
E = 3 * D
T = 2048  # tokens per core shard
CH = 512  # psum chunk width
ND = D // 128  # contraction tiles
NC = D // 128  # channel tiles
NCH = T // CH  # chunks per shard


def build_program():
    """One-core program; SPMD across cores with different data."""
    nc = bacc.Bacc(None)
    xT = nc.dram_tensor("xT", [D, T], F16, kind="ExternalInput")
    w1T = nc.dram_tensor("w1T", [D, E], F16, kind="ExternalInput")
    w2T = nc.dram_tensor("w2T", [D, D], F16, kind="ExternalInput")
    cw = nc.dram_tensor("cw", [D, KS], F32, kind="ExternalInput")
    gh = nc.dram_tensor("gh", [D, KS - 1], F16, kind="ExternalInput")
    outT = nc.dram_tensor("outT", [D, T], F32, kind="ExternalOutput")

    def w_batch_src(w, ncols, e, g):
        # [128 part, KG ksub, 128 m] gather of KG stacked [128,128] tiles:
        # element (p, ks, m) = w[(g*KG+ks)*128 + p, e*128 + m]
        off = (g * KG * 128) * ncols + e * 128
        return bass.AP(w, off, [[ncols, 128], [128 * ncols, KG], [1, 128]])

    with tile.TileContext(nc) as tc, ExitStack() as ctx:
        wp = ctx.enter_context(tc.tile_pool(name="wp", bufs=24))
        w2p = ctx.enter_context(tc.tile_pool(name="w2p", bufs=8))
        xp = ctx.enter_context(tc.tile_pool(name="xp", bufs=1))
        rp = ctx.enter_context(tc.tile_pool(name="rp", bufs=1))
        gwp = ctx.enter_context(tc.tile_pool(name="gwp", bufs=3))
        scrp = ctx.enter_context(tc.tile_pool(name="scrp", bufs=3))
        tmpp = ctx.enter_context(tc.tile_pool(name="tmpp", bufs=4))
        stgp = ctx.enter_context(tc.tile_pool(name="stgp", bufs=4))
        smallp = ctx.enter_context(tc.tile_pool(name="smallp", bufs=1))
        psp = ctx.enter_context(tc.tile_pool(name="psp", bufs=8, space="PSUM"))

        # persistent small tiles: conv weights (GpSimd queue: off the
        # critical weight/x streams)
        cwt = []
        for c in range(NC):
            t = smallp.tile([128, KS], F32, tag=f"cw{c}", name=f"cw{c}")
            nc.gpsimd.dma_start(t[:], cw[c * 128 : (c + 1) * 128, :])
            cwt.append(t)

        # x resident in SBUF (fp16, 64KB/partition) on the Scalar queue,
        # gathered KG k-planes per DMA (few large issues -> fast startup),
        # split by 512-col chunk so chunk u=0 lands first.
        # xu[u][kk][:, j*CH:(j+1)*CH] holds x k-tile (kk*KG+j), cols of chunk u.
        xu = [
            [
                xp.tile([128, KG * CH], F16, tag=f"x{u}_{kk}", name=f"x{u}_{kk}")
                for kk in range(ND // KG)
            ]
            for u in range(NCH)
        ]
        def load_x_chunk(u, eng):
            for kk in range(ND // KG):
                srcap = bass.AP(
                    xT,
                    (kk * KG * 128) * T + u * CH,
                    [[T, 128], [128 * T, KG], [1, CH]],
                )
                eng.dma_start(
                    xu[u][kk][:].rearrange("p (g m) -> p g m", m=CH), srcap
                )

        # u0 first (PE-critical), u1/u3 behind it on the Scalar queue;
        # u2 goes on the Sync queue between c0's weight loads (see c-loop).
        for u in (0, 1, 3):
            load_x_chunk(u, nc.scalar)

        def load_w_tiles(pool, tag, w, ncols, e):
            tiles = []
            for g in range(ND // KG):
                wt = pool.tile([128, KG * 128], F16, tag=tag, name=f"{tag}_t")
                nc.sync.dma_start(
                    wt[:].rearrange("p (g m) -> p g m", m=128),
                    w_batch_src(w, ncols, e, g),
                )
                tiles.append(wt)
            return tiles

        def mm_accum(ps, wtiles, u):
            for g in range(ND // KG):
                for ks in range(KG):
                    k = g * KG + ks
                    nc.tensor.matmul(
                        ps[:],
                        wtiles[g][:, ks * 128 : (ks + 1) * 128],
                        xu[u][g][:, ks * CH : (ks + 1) * CH],
                        start=(k == 0),
                        stop=(k == ND - 1),
                    )

        Rt = []
        for c in range(NC):
            wB = load_w_tiles(wp, "w1", w1T, E, c)
            wX = load_w_tiles(wp, "w1", w1T, E, 2 * NC + c)
            if c == 0:
                # c0 streams x while computing: u2 rides the Sync queue
                # between the weight loads it is needed after.
                load_x_chunk(2, nc.sync)
            wC = load_w_tiles(wp, "w1", w1T, E, NC + c)
            gw = gwp.tile([128, T + KS - 1], F16, tag="gw", name=f"gw{c}")
            nc.gpsimd.dma_start(gw[:, 0 : KS - 1], gh[c * 128 : (c + 1) * 128, :])
            R = rp.tile([128, T], F16, tag=f"R{c}", name=f"R{c}")
            Rt.append(R)
            for u in range(NCH):
                psB = psp.tile([128, CH], F32, tag="ps", name=f"psB{c}_{u}")
                mm_accum(psB, wB, u)
                psX = psp.tile([128, CH], F32, tag="ps", name=f"psX{c}_{u}")
                mm_accum(psX, wX, u)
                # DVE reads at most one PSUM operand per instruction:
                # stage Bg into SBUF, then multiply with Xg.
                tmp = tmpp.tile([128, CH], F32, tag="tmp", name=f"tmp{c}_{u}")
                nc.vector.tensor_copy(tmp[:], psB[:])
                nc.vector.tensor_mul(
                    gw[:, KS - 1 + u * CH : KS - 1 + (u + 1) * CH], tmp[:], psX[:]
                )
                if c == 0:
                    # c0 only: chunk-major. Each x chunk feeds B, X and C
                    # back-to-back (x is still streaming in at ~PE rate),
                    # and conv + R-mul run per chunk so the psC bank frees
                    # immediately instead of waiting on a full-width conv.
                    psC = psp.tile([128, CH], F32, tag="ps", name=f"psC{c}_{u}")
                    mm_accum(psC, wC, u)
                    prev = None
                    for j in range(KS):
                        dst = scrp.tile(
                            [128, CH], F16, tag="scc", name=f"scc{c}_{u}_{j}"
                        )
                        src_ap = gw[:, u * CH + j : u * CH + j + CH]
                        if j == 0:
                            nc.vector.tensor_scalar_mul(
                                dst[:], src_ap, cwt[c][:, 0:1]
                            )
                        else:
                            nc.vector.scalar_tensor_tensor(
                                dst[:],
                                src_ap,
                                cwt[c][:, j : j + 1],
                                prev[:],
                                mybir.AluOpType.mult,
                                mybir.AluOpType.add,
                            )
                        prev = dst
                    nc.vector.tensor_mul(
                        R[:, u * CH : (u + 1) * CH], prev[:], psC[:]
                    )
            if c == 0:
                continue
            # depthwise causal conv over gw -> s (fp16, full width)
            s = scrp.tile([128, T], F16, tag="scr", name=f"s0_{c}")
            nc.vector.tensor_scalar_mul(s[:], gw[:, 0:T], cwt[c][:, 0:1])
            for j in range(1, KS):
                s2 = scrp.tile([128, T], F16, tag="scr", name=f"s{j}_{c}")
                nc.vector.scalar_tensor_tensor(
                    s2[:],
                    gw[:, j : j + T],
                    cwt[c][:, j : j + 1],
                    s[:],
                    mybir.AluOpType.mult,
                    mybir.AluOpType.add,
                )
                s = s2
            # R = Cg * conv
            for u in range(NCH):
                psC = psp.tile([128, CH], F32, tag="ps", name=f"psC{c}_{u}")
                mm_accum(psC, wC, u)
                nc.vector.tensor_mul(
                    R[:, u * CH : (u + 1) * CH],
                    s[:, u * CH : (u + 1) * CH],
                    psC[:],
                )

        # ---- mm2: out = R.T @ w2.T (channel-major) ----
        for f in range(NC):
            w2t = load_w_tiles(w2p, "w2", w2T, D, f)
            for u in range(NCH):
                ps2 = psp.tile([128, CH], F32, tag="ps", name=f"ps2{f}_{u}")
                for g in range(NC // KG):
                    for cs in range(KG):
                        c = g * KG + cs
                        nc.tensor.matmul(
                            ps2[:],
                            w2t[g][:, cs * 128 : (cs + 1) * 128],
                            Rt[c][:, u * CH : (u + 1) * CH],
                            start=(c == 0),
                            stop=(c == NC - 1),
                        )
                st = stgp.tile([128, CH], F32, tag="stg", name=f"st{f}_{u}")
                nc.vector.tensor_copy(st[:], ps2[:])
                # alternate store queues: halves tail latency and sync-queue
                # contention with the w2 stream
                seng = nc.sync if (f * NCH + u) % 2 == 0 else nc.scalar
                seng.dma_start(
                    outT[f * 128 : (f + 1) * 128, u * CH : (u + 1) * CH],
                    st[:],
                )

    nc.finalize()
    return nc


def shard_inputs(x, w1, w2, conv_w):
    """Full inputs -> per-core in_maps (channel-major fp16 device layouts)."""
    B, S, _ = x.shape
    n_shards = (B * S) // T
    w1T = np.ascontiguousarray(w1.T).astype(np.float16)
    w2T = np.ascontiguousarray(w2.T).astype(np.float16)
    cwh = np.ascontiguousarray(conv_w[:, 0, :]).astype(np.float32)
    shards_per_batch = S // T
    in_maps = []
    for s in range(n_shards):
        b, h = divmod(s, shards_per_batch)
        xs = x[b, h * T : (h + 1) * T, :]
        xTs = np.ascontiguousarray(xs.T).astype(np.float16)
        if h == 0:
            ghs = np.zeros((D, KS - 1), np.float16)
        else:
            xh = x[b, h * T - (KS - 1) : h * T, :]
            Bg = xh @ w1[0:D].T
            Xg = xh @ w1[2 * D : 3 * D].T
            ghs = np.ascontiguousarray((Bg * Xg).T).astype(np.float16)
        in_maps.append({"xT": xTs, "w1T": w1T, "w2T": w2T, "cw": cwh, "gh": ghs})
    return in_maps


_PROGRAM_CACHE = {}


def run(x, w1, w2, conv_w, trace=False, **_ignored):
    B, S, _ = x.shape
    if "prog" not in _PROGRAM_CACHE:
        _PROGRAM_CACHE["prog"] = build_program()
    nc = _PROGRAM_CACHE["prog"]
    in_maps = shard_inputs(x, w1, w2, conv_w)
    n_shards = len(in_maps)
    res = run_bass_kernel_spmd(nc, in_maps, core_ids=list(range(n_shards)), trace=trace)
    shards_per_batch = S // T
    out = np.empty((B, S, D), np.float32)
    for s in range(n_shards):
        b, h = divmod(s, shards_per_batch)
        out[b, h * T : (h + 1) * T, :] = res.results[s]["outT"].T
    return out, res


def kernel(x, w1, w2, conv_w):
    x = np.asarray(x, np.float32)
    w1 = np.asarray(w1, np.float32)
    w2 = np.asarray(w2, np.float32)
    conv_w = np.asarray(conv_w, np.float32)
    out, _ = run(x, w1, w2, conv_w)
    return out


# revision 23
# speedup vs baseline: 1.1051x; 1.0108x over previous
"""Trainium2 Bass kernel for nn_GatedShortBlock (gated depthwise-conv block).

Math (per batch b):
  BCx = x @ w1.T ; Bg, Cg, Xg = split(BCx, 3)
  gated = Bg * Xg
  conv  = causal depthwise conv1d(gated, conv_w, K=4)  (left pad 3)
  out   = (Cg * conv) @ w2.T

Sharding: data-parallel over (batch, seq-half) -> 8 shards of 2048 tokens.
Each core computes its shard fully on-device in a channel-major (transposed)
layout; the 3-token causal halo of `gated` at each shard start is computed
on the host (tiny) and shipped as an input.

v2: fp16 operands (full PE rate, ~6e-4 rel err), single pass over w1/w2
(each weight tile is loaded once and reused for all 4 token chunks),
whole 2048-token shard processed as one block.
"""

import sys

sys.path.insert(0, "/opt/trn_rl_repo")

import numpy as np
from contextlib import ExitStack

import concourse.bass as bass
import concourse.tile as tile
from concourse import bacc, mybir
from concourse.bass_utils import run_bass_kernel_spmd

F32 = mybir.dt.float32
F16 = mybir.dt.float16
KS = 4  # conv kernel size
KG = 4  # k-subtiles batched per weight DMA

D = 2048
E = 3 * D
T = 2048  # tokens per core shard
CH = 512  # psum chunk width
ND = D // 128  # contraction tiles
NC = D // 128  # channel tiles
NCH = T // CH  # chunks per shard


def build_program():
    """One-core program; SPMD across cores with different data."""
    nc = bacc.Bacc(None)
    xT = nc.dram_tensor("xT", [D, T], F16, kind="ExternalInput")
    w1T = nc.dram_tensor("w1T", [D, E], F16, kind="ExternalInput")
    w2T = nc.dram_tensor("w2T", [D, D], F16, kind="ExternalInput")
    cw = nc.dram_tensor("cw", [D, KS], F32, kind="ExternalInput")
    gh = nc.dram_tensor("gh", [D, KS - 1], F16, kind="ExternalInput")
    outT = nc.dram_tensor("outT", [D, T], F32, kind="ExternalOutput")

    def w_batch_src(w, ncols, e, g):
        # [128 part, KG ksub, 128 m] gather of KG stacked [128,128] tiles:
        # element (p, ks, m) = w[(g*KG+ks)*128 + p, e*128 + m]
        off = (g * KG * 128) * ncols + e * 128
        return bass.AP(w, off, [[ncols, 128], [128 * ncols, KG], [1, 128]])

    with tile.TileContext(nc) as tc, ExitStack() as ctx:
        wp = ctx.enter_context(tc.tile_pool(name="wp", bufs=24))
        w2p = ctx.enter_context(tc.tile_pool(name="w2p", bufs=8))
        xp = ctx.enter_context(tc.tile_pool(name="xp", bufs=1))
        rp = ctx.enter_context(tc.tile_pool(name="rp", bufs=1))
        gwp = ctx.enter_context(tc.tile_pool(name="gwp", bufs=3))
        scrp = ctx.enter_context(tc.tile_pool(name="scrp", bufs=3))
        tmpp = ctx.enter_context(tc.tile_pool(name="tmpp", bufs=4))
        stgp = ctx.enter_context(tc.tile_pool(name="stgp", bufs=4))
        smallp = ctx.enter_context(tc.tile_pool(name="smallp", bufs=1))
        psp = ctx.enter_context(tc.tile_pool(name="psp", bufs=8, space="PSUM"))

        # persistent small tiles: conv weights (GpSimd queue: off the
        # critical weight/x streams)
        cwt = []
        for c in range(NC):
            t = smallp.tile([128, KS], F32, tag=f"cw{c}", name=f"cw{c}")
            nc.gpsimd.dma_start(t[:], cw[c * 128 : (c + 1) * 128, :])
            cwt.append(t)

        # x resident in SBUF (fp16, 64KB/partition) on the Scalar queue,
        # gathered KG k-planes per DMA (few large issues -> fast startup),
        # split by 512-col chunk so chunk u=0 lands first.
        # xu[u][kk][:, j*CH:(j+1)*CH] holds x k-tile (kk*KG+j), cols of chunk u.
        xu = [
            [
                xp.tile([128, KG * CH], F16, tag=f"x{u}_{kk}", name=f"x{u}_{kk}")
                for kk in range(ND // KG)
            ]
            for u in range(NCH)
        ]
        def load_x_chunk(u, eng):
            for kk in range(ND // KG):
                srcap = bass.AP(
                    xT,
                    (kk * KG * 128) * T + u * CH,
                    [[T, 128], [128 * T, KG], [1, CH]],
                )
                eng.dma_start(
                    xu[u][kk][:].rearrange("p (g m) -> p g m", m=CH), srcap
                )

        # u0 first (PE-critical), u1/u3 behind it on the Scalar queue;
        # u2 goes on the Sync queue between c0's weight loads (see c-loop).
        for u in (0, 1, 3):
            load_x_chunk(u, nc.scalar)

        def load_w_tiles(pool, tag, w, ncols, e):
            tiles = []
            for g in range(ND // KG):
                wt = pool.tile([128, KG * 128], F16, tag=tag, name=f"{tag}_t")
                nc.sync.dma_start(
                    wt[:].rearrange("p (g m) -> p g m", m=128),
                    w_batch_src(w, ncols, e, g),
                )
                tiles.append(wt)
            return tiles

        def mm_accum(ps, wtiles, u):
            for g in range(ND // KG):
                for ks in range(KG):
                    k = g * KG + ks
                    nc.tensor.matmul(
                        ps[:],
                        wtiles[g][:, ks * 128 : (ks + 1) * 128],
                        xu[u][g][:, ks * CH : (ks + 1) * CH],
                        start=(k == 0),
                        stop=(k == ND - 1),
                    )

        Rt = []
        for c in range(NC):
            wB = load_w_tiles(wp, "w1", w1T, E, c)
            wX = load_w_tiles(wp, "w1", w1T, E, 2 * NC + c)
            wC = load_w_tiles(wp, "w1", w1T, E, NC + c)
            if c == 0:
                # c0 streams x while computing: u2 rides the Sync queue
                # AFTER all c0 weights (wC is needed at ~+7us in the
                # chunk-major c0 body; u2 not until ~+21us).
                load_x_chunk(2, nc.sync)
            gw = gwp.tile([128, T + KS - 1], F16, tag="gw", name=f"gw{c}")
            nc.gpsimd.dma_start(gw[:, 0 : KS - 1], gh[c * 128 : (c + 1) * 128, :])
            R = rp.tile([128, T], F16, tag=f"R{c}", name=f"R{c}")
            Rt.append(R)
            for u in range(NCH):
                psB = psp.tile([128, CH], F32, tag="ps", name=f"psB{c}_{u}")
                mm_accum(psB, wB, u)
                psX = psp.tile([128, CH], F32, tag="ps", name=f"psX{c}_{u}")
                mm_accum(psX, wX, u)
                # DVE reads at most one PSUM operand per instruction:
                # stage Bg into SBUF, then multiply with Xg.
                tmp = tmpp.tile([128, CH], F32, tag="tmp", name=f"tmp{c}_{u}")
                nc.vector.tensor_copy(tmp[:], psB[:])
                nc.vector.tensor_mul(
                    gw[:, KS - 1 + u * CH : KS - 1 + (u + 1) * CH], tmp[:], psX[:]
                )
                if c == 0:
                    # c0 only: chunk-major. Each x chunk feeds B, X and C
                    # back-to-back (x is still streaming in at ~PE rate),
                    # and conv + R-mul run per chunk so the psC bank frees
                    # immediately instead of waiting on a full-width conv.
                    psC = psp.tile([128, CH], F32, tag="ps", name=f"psC{c}_{u}")
                    mm_accum(psC, wC, u)
                    prev = None
                    for j in range(KS):
                        dst = scrp.tile(
                            [128, CH], F16, tag="scc", name=f"scc{c}_{u}_{j}"
                        )
                        src_ap = gw[:, u * CH + j : u * CH + j + CH]
                        if j == 0:
                            nc.vector.tensor_scalar_mul(
                                dst[:], src_ap, cwt[c][:, 0:1]
                            )
                        else:
                            nc.vector.scalar_tensor_tensor(
                                dst[:],
                                src_ap,
                                cwt[c][:, j : j + 1],
                                prev[:],
                                mybir.AluOpType.mult,
                                mybir.AluOpType.add,
                            )
                        prev = dst
                    nc.vector.tensor_mul(
                        R[:, u * CH : (u + 1) * CH], prev[:], psC[:]
                    )
            if c == 0:
                continue
            # depthwise causal conv over gw -> s (fp16, full width)
            s = scrp.tile([128, T], F16, tag="scr", name=f"s0_{c}")
            nc.vector.tensor_scalar_mul(s[:], gw[:, 0:T], cwt[c][:, 0:1])
            for j in range(1, KS):
                s2 = scrp.tile([128, T], F16, tag="scr", name=f"s{j}_{c}")
                nc.vector.scalar_tensor_tensor(
                    s2[:],
                    gw[:, j : j + T],
                    cwt[c][:, j : j + 1],
                    s[:],
                    mybir.AluOpType.mult,
                    mybir.AluOpType.add,
                )
                s = s2
            # R = Cg * conv
            for u in range(NCH):
                psC = psp.tile([128, CH], F32, tag="ps", name=f"psC{c}_{u}")
                mm_accum(psC, wC, u)
                nc.vector.tensor_mul(
                    R[:, u * CH : (u + 1) * CH],
                    s[:, u * CH : (u + 1) * CH],
                    psC[:],
                )

        # ---- mm2: out = R.T @ w2.T (channel-major) ----
        for f in range(NC):
            w2t = load_w_tiles(w2p, "w2", w2T, D, f)
            for u in range(NCH):
                ps2 = psp.tile([128, CH], F32, tag="ps", name=f"ps2{f}_{u}")
                for g in range(NC // KG):
                    for cs in range(KG):
                        c = g * KG + cs
                        nc.tensor.matmul(
                            ps2[:],
                            w2t[g][:, cs * 128 : (cs + 1) * 128],
                            Rt[c][:, u * CH : (u + 1) * CH],
                            start=(c == 0),
                            stop=(c == NC - 1),
                        )
                st = stgp.tile([128, CH], F32, tag="stg", name=f"st{f}_{u}")
                nc.vector.tensor_copy(st[:], ps2[:])
                # alternate store queues: halves tail latency and sync-queue
                # contention with the w2 stream
                seng = nc.sync if (f * NCH + u) % 2 == 0 else nc.scalar
                seng.dma_start(
                    outT[f * 128 : (f + 1) * 128, u * CH : (u + 1) * CH],
                    st[:],
                )

    nc.finalize()
    return nc


def shard_inputs(x, w1, w2, conv_w):
    """Full inputs -> per-core in_maps (channel-major fp16 device layouts)."""
    B, S, _ = x.shape
    n_shards = (B * S) // T
    w1T = np.ascontiguousarray(w1.T).astype(np.float16)
    w2T = np.ascontiguousarray(w2.T).astype(np.float16)
    cwh = np.ascontiguousarray(conv_w[:, 0, :]).astype(np.float32)
    shards_per_batch = S // T
    in_maps = []
    for s in range(n_shards):
        b, h = divmod(s, shards_per_batch)
        xs = x[b, h * T : (h + 1) * T, :]
        xTs = np.ascontiguousarray(xs.T).astype(np.float16)
        if h == 0:
            ghs = np.zeros((D, KS - 1), np.float16)
        else:
            xh = x[b, h * T - (KS - 1) : h * T, :]
            Bg = xh @ w1[0:D].T
            Xg = xh @ w1[2 * D : 3 * D].T
            ghs = np.ascontiguousarray((Bg * Xg).T).astype(np.float16)
        in_maps.append({"xT": xTs, "w1T": w1T, "w2T": w2T, "cw": cwh, "gh": ghs})
    return in_maps


_PROGRAM_CACHE = {}


def run(x, w1, w2, conv_w, trace=False, **_ignored):
    B, S, _ = x.shape
    if "prog" not in _PROGRAM_CACHE:
        _PROGRAM_CACHE["prog"] = build_program()
    nc = _PROGRAM_CACHE["prog"]
    in_maps = shard_inputs(x, w1, w2, conv_w)
    n_shards = len(in_maps)
    res = run_bass_kernel_spmd(nc, in_maps, core_ids=list(range(n_shards)), trace=trace)
    shards_per_batch = S // T
    out = np.empty((B, S, D), np.float32)
    for s in range(n_shards):
        b, h = divmod(s, shards_per_batch)
        out[b, h * T : (h + 1) * T, :] = res.results[s]["outT"].T
    return out, res


def kernel(x, w1, w2, conv_w):
    x = np.asarray(x, np.float32)
    w1 = np.asarray(w1, np.float32)
    w2 = np.asarray(w2, np.float32)
    conv_w = np.asarray(conv_w, np.float32)
    out, _ = run(x, w1, w2, conv_w)
    return out
